# revision 2
# baseline (speedup 1.0000x reference)
"""Causal self-attention Trainium2 kernel (8-core SPMD), v2.

Problem: x[4,2048,1024] @ w_qkv[1024,3072] -> per-head causal attention
(16 heads, hd=64) -> ctx @ w_out[1024,1024].

Sharding (8 cores): core c handles batch b = c//2 and head-group
g = c%2 (8 heads). Each core computes a partial output; host sums the
two partials per batch (tensor-parallel row-split of w_out).

v2 changes vs v1:
- Host passes x pre-transposed (xT, bf16) and bf16 weights: eliminates
  all 128 PE transposes, the f32->bf16 conversion copies, and halves
  input DMA bytes. DMA lands directly in persistent SBUF tiles.
- bf16 output (host upcasts + sums partial pairs): halves output DMA.
- Emission interleaves projection waves into the attention t-unit loop
  so PE has filler work while ACT (exp) is the per-unit bottleneck.

Device algorithm (per core), all matmuls bf16 with fp32 PSUM:
  qk proj:   qkT[:, tq, c] = (wqk tile)^T xT  (computed transposed)
  v proj:    vaug[:, h, si, 0:64] = (xT tile)^T wv  (ones-augmented)
  attention (query-block j, head pair t; pairs row-packed K=64):
    scoresT[sk,sq] = k_h^T q_h     (tile_position row packing)
    expT = exp(scale*scoresT)      (ACT; causal diag masked via bf16 mul)
    ctxT_aug[128,sq] = [v_h | 1]^T @ expT  (rows 0:64 ctx, 64:128 sums)
    ctxT = ctxT_aug[0:64] * recip(ctxT_aug[64:128])
  out proj:  out rows = ctxT^T @ w_out_rows  (partial; host reduces)
"""

import threading

import numpy as np

S = 2048
D = 1024
B = 4
NCORES = 8
ST = 128           # seq tile (partitions)
NS = S // ST       # 16
SQ = 512           # query-block width (matmul free dim)
NJ = S // SQ       # 4
ND = D // 128      # 8 contraction tiles
NPAIR = 4          # head pairs per core
SCALE = 0.125      # 1/sqrt(64)

_cache = {}
_lock = threading.Lock()


def build_nc(reps=1):
    from contextlib import ExitStack, nullcontext

    import concourse.mybir as mybir
    import concourse.tile as tile
    from concourse import bacc

    f32 = mybir.dt.float32
    bf16 = mybir.dt.bfloat16

    nc = bacc.Bacc("TRN2", target_bir_lowering=False, debug=False)

    xT = nc.dram_tensor("xT", [D, S], bf16, kind="ExternalInput").ap()
    wqk = nc.dram_tensor("wqk", [D, 1024], bf16, kind="ExternalInput").ap()
    wv = nc.dram_tensor("wv", [D, 512], bf16, kind="ExternalInput").ap()
    wout = nc.dram_tensor("wout", [512, D], bf16, kind="ExternalInput").ap()
    out = nc.dram_tensor("out", [S, D], bf16, kind="ExternalOutput").ap()

    with ExitStack() as ctx:
        tc = ctx.enter_context(tile.TileContext(nc))
        const = ctx.enter_context(tc.tile_pool(name="const", bufs=1))
        persist = ctx.enter_context(tc.tile_pool(name="persist", bufs=1))
        expp = ctx.enter_context(tc.tile_pool(name="expp", bufs=6))
        recp = ctx.enter_context(tc.tile_pool(name="recp", bufs=2))

        # Diagonal causal mask for (sq=512)-wide exp tiles holding two
        # 128-row sk blocks: mask[p, w, c] = 1 if c - p - 128*w >= 0.
        m01 = const.tile([128, 2, SQ], bf16)
        nc.vector.memset(m01, 1.0)
        nc.gpsimd.affine_select(
            out=m01, in_=m01, compare_op=mybir.AluOpType.is_ge, fill=0.0,
            base=0, channel_multiplier=-1, pattern=[[-128, 2], [1, SQ]],
        )

        # --- persistent tensors ---
        qkT = persist.tile([128, 8, S], bf16)            # tiles 0-3 q pairs, 4-7 k
        vaug = persist.tile([128, 8, NS, 128], bf16)     # per head: [v | ones]
        ctxT = persist.tile([128, NPAIR, S], bf16)       # normalized ctx^T
        xT_sb = persist.tile([128, ND, S], bf16)         # x^T, d on partitions
        wqk_sb = persist.tile([128, ND, 1024], bf16)
        wv_sb = persist.tile([128, ND, 512], bf16)
        wout_sb = persist.tile([128, NPAIR, D], bf16)

        nc.vector.memset(vaug, 1.0)  # ones columns; v halves overwritten below
        # prime the ACT exp table set so the ~2.7us table load hides
        # under the projection phase instead of delaying attention
        warm = const.tile([128, 1], f32)
        nc.vector.memset(warm, 0.0)
        nc.scalar.activation(warm, warm,
                             mybir.ActivationFunctionType.Exp, scale=1.0)

        # repeat body for steady-state timing (reps>1: timing builds only)
        with (tc.For_i(0, reps, 1) if reps > 1 else nullcontext()):
            with (
                tc.tile_pool(name="wps", bufs=2, space="PSUM") as wps,
                tc.tile_pool(name="atps", bufs=2, space="PSUM") as atps,
                tc.tile_pool(name="outp", bufs=2) as outp,
            ):
                # --- input DMAs straight into persistent SBUF ---
                for ki in range(ND):
                    r = slice(128 * ki, 128 * ki + 128)
                    nc.sync.dma_start(xT_sb[:, ki, :], xT[r, :])
                    nc.sync.dma_start(wqk_sb[:, ki, :], wqk[r, :])
                    nc.sync.dma_start(wv_sb[:, ki, :], wv[r, :])
                for t in range(NPAIR):
                    r = slice(128 * t, 128 * t + 128)
                    nc.sync.dma_start(wout_sb[:, t, :], wout[r, :])

                def qk_proj(tq, jj):
                    c = slice(SQ * jj, SQ * jj + SQ)
                    ps = wps.tile([128, SQ], f32, tag="ps")
                    for ki in range(ND):
                        nc.tensor.matmul(
                            ps, wqk_sb[:, ki, 128 * tq:128 * tq + 128],
                            xT_sb[:, ki, c],
                            start=(ki == 0), stop=(ki == ND - 1),
                        )
                    nc.vector.tensor_copy(qkT[:, tq, c], ps)

                def v_proj(si):
                    r = slice(128 * si, 128 * si + 128)
                    psv = wps.tile([128, SQ], f32, tag="ps")
                    for ki in range(ND):
                        nc.tensor.matmul(
                            psv, xT_sb[:, ki, r], wv_sb[:, ki, :],
                            start=(ki == 0), stop=(ki == ND - 1),
                        )
                    nc.vector.tensor_copy(
                        vaug[:, 0:8, si, 0:64],
                        psv.rearrange("p (h e) -> p h e", h=8),
                    )

                def attn_core(j, fillers=()):
                    fillers = list(fillers)
                    c = slice(SQ * j, SQ * j + SQ)
                    nblk = 4 * j + 4
                    for t in range(NPAIR):
                        hA, hB = 2 * t, 2 * t + 1
                        psCA = atps.tile([128, SQ], f32, tag="ctx")
                        psCB = atps.tile([128, SQ], f32, tag="ctx")
                        for ip in range(nblk // 2):
                            # hi pair (sk offsets 256/384 into the query
                            # block): valid region is columns 256:512 only
                            hi = (2 * ip == 4 * j + 2)
                            co = 256 if hi else 0       # column offset
                            cw = SQ - co                # width
                            cq = slice(SQ * j + co, SQ * j + SQ)
                            psSA = atps.tile([128, 2, SQ], f32, tag="score")
                            psSB = atps.tile([128, 2, SQ], f32, tag="score")
                            for w in range(2):
                                i = 2 * ip + w
                                ks = slice(128 * i, 128 * i + 128)
                                nc.tensor.matmul(
                                    psSA[:, w, 0:cw], qkT[0:64, 4 + t, ks],
                                    qkT[0:64, t, cq], start=True, stop=True,
                                    tile_position=(0, 0),
                                )
                                nc.tensor.matmul(
                                    psSB[:, w, 0:cw], qkT[64:128, 4 + t, ks],
                                    qkT[64:128, t, cq], start=True, stop=True,
                                    tile_position=(64, 0),
                                )
                            expA = expp.tile([128, 2, SQ], bf16, tag="exp")
                            expB = expp.tile([128, 2, SQ], bf16, tag="exp")
                            nc.scalar.activation(
                                expA[:, :, 0:cw], psSA[:, :, 0:cw],
                                mybir.ActivationFunctionType.Exp,
                                scale=SCALE,
                            )
                            nc.scalar.activation(
                                expB[:, :, 0:cw], psSB[:, :, 0:cw],
                                mybir.ActivationFunctionType.Exp,
                                scale=SCALE,
                            )
                            if 2 * ip >= 4 * j:  # diagonal pair: causal mask
                                m = m01[:, :, 0:cw] if hi else m01
                                nc.vector.tensor_mul(
                                    expA[:, :, 0:cw], expA[:, :, 0:cw], m
                                )
                                nc.vector.tensor_mul(
                                    expB[:, :, 0:cw], expB[:, :, 0:cw], m
                                )
                            for w in range(2):
                                i = 2 * ip + w
                                nc.tensor.matmul(
                                    psCA[:, co:SQ], vaug[:, hA, i, :],
                                    expA[:, w, 0:cw],
                                    start=(i == 0), stop=(i == nblk - 1),
                                )
                                nc.tensor.matmul(
                                    psCB[:, co:SQ], vaug[:, hB, i, :],
                                    expB[:, w, 0:cw],
                                    start=(i == 0), stop=(i == nblk - 1),
                                )
                        # normalize: ctx rows 0:64 / sums rows 64:128
                        recA = recp.tile([128, SQ], f32, tag="rec")
                        recB = recp.tile([128, SQ], f32, tag="rec")
                        nc.vector.reciprocal(recA[64:128, :], psCA[64:128, :])
                        nc.vector.tensor_mul(
                            ctxT[0:64, t, c], psCA[0:64, :], recA[64:128, :]
                        )
                        nc.vector.reciprocal(recB[64:128, :], psCB[64:128, :])
                        nc.vector.tensor_mul(
                            ctxT[64:128, t, c], psCB[0:64, :], recB[64:128, :]
                        )
                        # PE filler work for the ACT-bound stretches
                        if fillers:
                            fillers.pop(0)()
                    for f in fillers:
                        f()

                def out_proj(j):
                    for si in range(4 * j, 4 * j + 4):
                        r = slice(128 * si, 128 * si + 128)
                        o_st = outp.tile([128, D], bf16, tag="o_st")
                        for n in range(2):
                            pso = wps.tile([128, 512], f32, tag="ps")
                            for t in range(NPAIR):
                                nc.tensor.matmul(
                                    pso, ctxT[:, t, r],
                                    wout_sb[:, t, 512 * n:512 * n + 512],
                                    start=(t == 0), stop=(t == NPAIR - 1),
                                )
                            nc.vector.tensor_copy(
                                o_st[:, 512 * n:512 * n + 512], pso
                            )
                        nc.sync.dma_start(out[r, :], o_st)

                TQ_ORDER = (0, 4, 1, 5, 2, 6, 3, 7)

                def qk_pairf(jj, a, b):
                    def f():
                        qk_proj(a, jj)
                        qk_proj(b, jj)
                    return f

                # wave 0: q/k cols for blocks 0-1 come first so attention
                # t-pairs unblock early; v for s-tiles 0-3
                for tq in TQ_ORDER:
                    qk_proj(tq, 0)
                for si in range(0, 4):
                    v_proj(si)
                attn_core(0, [qk_pairf(1, 0, 4), qk_pairf(1, 1, 5),
                              qk_pairf(1, 2, 6), qk_pairf(1, 3, 7)])
                for si in range(4, 8):
                    v_proj(si)
                attn_core(1, [qk_pairf(2, 0, 4), qk_pairf(2, 1, 5),
                              qk_pairf(2, 2, 6), qk_pairf(2, 3, 7)])
                out_proj(0)
                for si in range(8, 12):
                    v_proj(si)
                attn_core(2, [qk_pairf(3, 0, 4), qk_pairf(3, 1, 5),
                              qk_pairf(3, 2, 6), qk_pairf(3, 3, 7)])
                out_proj(1)
                for si in range(12, 16):
                    v_proj(si)
                attn_core(3, [lambda: out_proj(2)])
                out_proj(3)

    if not nc.is_finalized():
        nc.finalize()
    return nc


def make_in_maps(x, w_qkv, w_out):
    import ml_dtypes

    bf16 = ml_dtypes.bfloat16
    x = np.asarray(x, dtype=np.float32)
    w_qkv = np.asarray(w_qkv, dtype=np.float32)
    w_out = np.asarray(w_out, dtype=np.float32)
    xT = [np.ascontiguousarray(x[b].T).astype(bf16) for b in range(B)]
    in_maps = []
    for c in range(NCORES):
        b, g = c // 2, c % 2
        cs = slice(512 * g, 512 * g + 512)
        in_maps.append({
            "xT": xT[b],
            "wqk": np.ascontiguousarray(
                np.concatenate([w_qkv[:, 512 * g:512 * g + 512],
                                w_qkv[:, 1024 + 512 * g:1024 + 512 * g + 512]],
                               axis=1)).astype(bf16),
            "wv": np.ascontiguousarray(
                w_qkv[:, 2048 + 512 * g:2048 + 512 * g + 512]).astype(bf16),
            "wout": np.ascontiguousarray(w_out[cs, :]).astype(bf16),
        })
    return in_maps


def run_sharded(inputs, trace=False, trace_kwargs=None):
    """Run on 8 neuron cores; returns (out[B,S,D], BassKernelResults)."""
    from concourse import bass_utils

    with _lock:
        if "nc" not in _cache:
            _cache["nc"] = build_nc()
    nc = _cache["nc"]
    in_maps = make_in_maps(**inputs)
    res = bass_utils.run_bass_kernel_spmd(
        nc, in_maps, core_ids=list(range(NCORES)),
        trace=trace, **(trace_kwargs or {}),
    )
    outs = np.stack(
        [res.results[2 * b]["out"].astype(np.float32)
         + res.results[2 * b + 1]["out"].astype(np.float32)
         for b in range(B)]
    )
    return outs, res


def kernel(x, w_qkv, w_out):
    out, _ = run_sharded({"x": x, "w_qkv": w_qkv, "w_out": w_out})
    return out


# revision 4
# speedup vs baseline: 56.0419x; 56.0419x over previous
"""Causal self-attention Trainium2 kernel (8-core SPMD), v2.

Problem: x[4,2048,1024] @ w_qkv[1024,3072] -> per-head causal attention
(16 heads, hd=64) -> ctx @ w_out[1024,1024].

Sharding (8 cores): core c handles batch b = c//2 and head-group
g = c%2 (8 heads). Each core computes a partial output; host sums the
two partials per batch (tensor-parallel row-split of w_out).

v2 changes vs v1:
- Host passes x pre-transposed (xT, bf16) and bf16 weights: eliminates
  all 128 PE transposes, the f32->bf16 conversion copies, and halves
  input DMA bytes. DMA lands directly in persistent SBUF tiles.
- bf16 output (host upcasts + sums partial pairs): halves output DMA.
- Emission interleaves projection waves into the attention t-unit loop
  so PE has filler work while ACT (exp) is the per-unit bottleneck.

Device algorithm (per core), all matmuls bf16 with fp32 PSUM:
  qk proj:   qkT[:, tq, c] = (wqk tile)^T xT  (computed transposed)
  v proj:    vaug[:, h, si, 0:64] = (xT tile)^T wv  (ones-augmented)
  attention (query-block j, head pair t; pairs row-packed K=64):
    scoresT[sk,sq] = k_h^T q_h     (tile_position row packing)
    expT = exp(scale*scoresT)      (ACT; causal diag masked via bf16 mul)
    ctxT_aug[128,sq] = [v_h | 1]^T @ expT  (rows 0:64 ctx, 64:128 sums)
    ctxT = ctxT_aug[0:64] * recip(ctxT_aug[64:128])
  out proj:  out rows = ctxT^T @ w_out_rows  (partial; host reduces)
"""

import threading

import numpy as np

S = 2048
D = 1024
B = 4
NCORES = 8
ST = 128           # seq tile (partitions)
NS = S // ST       # 16
SQ = 512           # query-block width (matmul free dim)
NJ = S // SQ       # 4
ND = D // 128      # 8 contraction tiles
NPAIR = 4          # head pairs per core
SCALE = 0.125      # 1/sqrt(64)

_cache = {}
_lock = threading.Lock()


def build_nc(reps=1):
    from contextlib import ExitStack, nullcontext

    import concourse.mybir as mybir
    import concourse.tile as tile
    from concourse import bacc

    f32 = mybir.dt.float32
    bf16 = mybir.dt.bfloat16

    nc = bacc.Bacc("TRN2", target_bir_lowering=False, debug=False)

    xT = nc.dram_tensor("xT", [D, S], bf16, kind="ExternalInput").ap()
    wqk = nc.dram_tensor("wqk", [D, 1024], bf16, kind="ExternalInput").ap()
    wv = nc.dram_tensor("wv", [D, 512], bf16, kind="ExternalInput").ap()
    wout = nc.dram_tensor("wout", [512, D], bf16, kind="ExternalInput").ap()
    out = nc.dram_tensor("out", [S, D], bf16, kind="ExternalOutput").ap()

    with ExitStack() as ctx:
        tc = ctx.enter_context(tile.TileContext(nc))
        const = ctx.enter_context(tc.tile_pool(name="const", bufs=1))
        persist = ctx.enter_context(tc.tile_pool(name="persist", bufs=1))
        expp = ctx.enter_context(tc.tile_pool(name="expp", bufs=6))
        recp = ctx.enter_context(tc.tile_pool(name="recp", bufs=2))

        # Triangle mask for the [128,128] on-diagonal blocks:
        # tri[p, c] = 1 if c >= p else 0.
        tri = const.tile([128, 128], bf16)
        nc.vector.memset(tri, 1.0)
        nc.gpsimd.affine_select(
            out=tri, in_=tri, compare_op=mybir.AluOpType.is_ge, fill=0.0,
            base=0, channel_multiplier=-1, pattern=[[1, 128]],
        )

        # --- persistent tensors ---
        qkT = persist.tile([128, 8, S], bf16)            # tiles 0-3 q pairs, 4-7 k
        vaug = persist.tile([128, 8, NS, 128], bf16)     # per head: [v | ones]
        ctxT = persist.tile([128, NPAIR, S], bf16)       # normalized ctx^T
        xT_sb = persist.tile([128, ND, S], bf16)         # x^T, d on partitions
        wqk_sb = persist.tile([128, ND, 1024], bf16)
        wv_sb = persist.tile([128, ND, 512], bf16)
        wout_sb = persist.tile([128, NPAIR, D], bf16)

        nc.vector.memset(vaug, 1.0)  # ones columns; v halves overwritten below
        # prime the ACT exp table set so the ~2.7us table load hides
        # under the projection phase instead of delaying attention
        warm = const.tile([128, 1], f32)
        nc.vector.memset(warm, 0.0)
        nc.scalar.activation(warm, warm,
                             mybir.ActivationFunctionType.Exp, scale=1.0)

        # repeat body for steady-state timing (reps>1: timing builds only)
        with (tc.For_i(0, reps, 1) if reps > 1 else nullcontext()):
            with (
                tc.tile_pool(name="wps", bufs=2, space="PSUM") as wps,
                tc.tile_pool(name="atps", bufs=2, space="PSUM") as atps,
                tc.tile_pool(name="outp", bufs=2) as outp,
            ):
                # --- input DMAs straight into persistent SBUF ---
                for ki in range(ND):
                    r = slice(128 * ki, 128 * ki + 128)
                    nc.sync.dma_start(xT_sb[:, ki, :], xT[r, :])
                    nc.sync.dma_start(wqk_sb[:, ki, :], wqk[r, :])
                    nc.sync.dma_start(wv_sb[:, ki, :], wv[r, :])
                for t in range(NPAIR):
                    r = slice(128 * t, 128 * t + 128)
                    nc.sync.dma_start(wout_sb[:, t, :], wout[r, :])

                def qk_proj(tq, jj):
                    c = slice(SQ * jj, SQ * jj + SQ)
                    ps = wps.tile([128, SQ], f32, tag="ps")
                    for ki in range(ND):
                        nc.tensor.matmul(
                            ps, wqk_sb[:, ki, 128 * tq:128 * tq + 128],
                            xT_sb[:, ki, c],
                            start=(ki == 0), stop=(ki == ND - 1),
                        )
                    nc.vector.tensor_copy(qkT[:, tq, c], ps)

                def v_proj(si):
                    r = slice(128 * si, 128 * si + 128)
                    psv = wps.tile([128, SQ], f32, tag="ps")
                    for ki in range(ND):
                        nc.tensor.matmul(
                            psv, xT_sb[:, ki, r], wv_sb[:, ki, :],
                            start=(ki == 0), stop=(ki == ND - 1),
                        )
                    nc.vector.tensor_copy(
                        vaug[:, 0:8, si, 0:64],
                        psv.rearrange("p (h e) -> p h e", h=8),
                    )

                def attn_core(j, fillers=()):
                    fillers = list(fillers)
                    c = slice(SQ * j, SQ * j + SQ)
                    nblk = 4 * j + 4
                    for t in range(NPAIR):
                        hA, hB = 2 * t, 2 * t + 1
                        psCA = atps.tile([128, SQ], f32, tag="ctx")
                        psCB = atps.tile([128, SQ], f32, tag="ctx")
                        for ip in range(nblk // 2):
                            # hi pair (sk offsets 256/384 into the query
                            # block): valid region is columns 256:512 only
                            hi = (2 * ip == 4 * j + 2)
                            co = 256 if hi else 0       # column offset
                            cw = SQ - co                # width
                            cq = slice(SQ * j + co, SQ * j + SQ)
                            psSA = atps.tile([128, 2, SQ], f32, tag="score")
                            psSB = atps.tile([128, 2, SQ], f32, tag="score")
                            for w in range(2):
                                i = 2 * ip + w
                                ks = slice(128 * i, 128 * i + 128)
                                nc.tensor.matmul(
                                    psSA[:, w, 0:cw], qkT[0:64, 4 + t, ks],
                                    qkT[0:64, t, cq], start=True, stop=True,
                                    tile_position=(0, 0),
                                )
                                nc.tensor.matmul(
                                    psSB[:, w, 0:cw], qkT[64:128, 4 + t, ks],
                                    qkT[64:128, t, cq], start=True, stop=True,
                                    tile_position=(64, 0),
                                )
                            expA = expp.tile([128, 2, SQ], bf16, tag="exp")
                            expB = expp.tile([128, 2, SQ], bf16, tag="exp")
                            nc.scalar.activation(
                                expA[:, :, 0:cw], psSA[:, :, 0:cw],
                                mybir.ActivationFunctionType.Exp,
                                scale=SCALE,
                            )
                            nc.scalar.activation(
                                expB[:, :, 0:cw], psSB[:, :, 0:cw],
                                mybir.ActivationFunctionType.Exp,
                                scale=SCALE,
                            )
                            diag = 2 * ip >= 4 * j
                            if diag:
                                # only the [128,128] on-diagonal block of
                                # each k-tile needs the triangle mask; the
                                # strictly-lower region is skipped by
                                # narrowing the ctx matmul instead
                                for e in (expA, expB):
                                    for w in range(2):
                                        nc.vector.tensor_mul(
                                            e[:, w, 128 * w:128 * w + 128],
                                            e[:, w, 128 * w:128 * w + 128],
                                            tri,
                                        )
                            for w in range(2):
                                i = 2 * ip + w
                                # causally-valid query columns for k-tile i
                                # start at 128*(i-4j) within the block
                                vo = 128 * (i - 4 * j) if diag else 0
                                nc.tensor.matmul(
                                    psCA[:, vo:SQ], vaug[:, hA, i, :],
                                    expA[:, w, vo - co:cw],
                                    start=(i == 0), stop=(i == nblk - 1),
                                )
                                nc.tensor.matmul(
                                    psCB[:, vo:SQ], vaug[:, hB, i, :],
                                    expB[:, w, vo - co:cw],
                                    start=(i == 0), stop=(i == nblk - 1),
                                )
                        # normalize: ctx rows 0:64 / sums rows 64:128
                        recA = recp.tile([128, SQ], f32, tag="rec")
                        recB = recp.tile([128, SQ], f32, tag="rec")
                        nc.vector.reciprocal(recA[64:128, :], psCA[64:128, :])
                        nc.vector.tensor_mul(
                            ctxT[0:64, t, c], psCA[0:64, :], recA[64:128, :]
                        )
                        nc.vector.reciprocal(recB[64:128, :], psCB[64:128, :])
                        nc.vector.tensor_mul(
                            ctxT[64:128, t, c], psCB[0:64, :], recB[64:128, :]
                        )
                        # PE filler work for the ACT-bound stretches
                        if fillers:
                            fillers.pop(0)()
                    for f in fillers:
                        f()

                def out_proj(j):
                    for si in range(4 * j, 4 * j + 4):
                        r = slice(128 * si, 128 * si + 128)
                        o_st = outp.tile([128, D], bf16, tag="o_st")
                        for n in range(2):
                            pso = wps.tile([128, 512], f32, tag="ps")
                            for t in range(NPAIR):
                                nc.tensor.matmul(
                                    pso, ctxT[:, t, r],
                                    wout_sb[:, t, 512 * n:512 * n + 512],
                                    start=(t == 0), stop=(t == NPAIR - 1),
                                )
                            nc.vector.tensor_copy(
                                o_st[:, 512 * n:512 * n + 512], pso
                            )
                        nc.sync.dma_start(out[r, :], o_st)

                TQ_ORDER = (0, 4, 1, 5, 2, 6, 3, 7)

                def qk_pairf(jj, a, b):
                    def f():
                        qk_proj(a, jj)
                        qk_proj(b, jj)
                    return f

                # wave 0: q/k cols for blocks 0-1 come first so attention
                # t-pairs unblock early; v for s-tiles 0-3
                for tq in TQ_ORDER:
                    qk_proj(tq, 0)
                for si in range(0, 4):
                    v_proj(si)
                attn_core(0, [qk_pairf(1, 0, 4), qk_pairf(1, 1, 5),
                              qk_pairf(1, 2, 6), qk_pairf(1, 3, 7)])
                for si in range(4, 8):
                    v_proj(si)
                attn_core(1, [qk_pairf(2, 0, 4), qk_pairf(2, 1, 5),
                              qk_pairf(2, 2, 6), qk_pairf(2, 3, 7)])
                out_proj(0)
                for si in range(8, 12):
                    v_proj(si)
                attn_core(2, [qk_pairf(3, 0, 4), qk_pairf(3, 1, 5),
                              qk_pairf(3, 2, 6), qk_pairf(3, 3, 7)])
                out_proj(1)
                for si in range(12, 16):
                    v_proj(si)
                attn_core(3, [lambda: out_proj(2)])
                out_proj(3)

    if not nc.is_finalized():
        nc.finalize()
    return nc


def make_in_maps(x, w_qkv, w_out):
    import ml_dtypes

    bf16 = ml_dtypes.bfloat16
    x = np.asarray(x, dtype=np.float32)
    w_qkv = np.asarray(w_qkv, dtype=np.float32)
    w_out = np.asarray(w_out, dtype=np.float32)
    xT = [np.ascontiguousarray(x[b].T).astype(bf16) for b in range(B)]
    in_maps = []
    for c in range(NCORES):
        b, g = c // 2, c % 2
        cs = slice(512 * g, 512 * g + 512)
        in_maps.append({
            "xT": xT[b],
            "wqk": np.ascontiguousarray(
                np.concatenate([w_qkv[:, 512 * g:512 * g + 512],
                                w_qkv[:, 1024 + 512 * g:1024 + 512 * g + 512]],
                               axis=1)).astype(bf16),
            "wv": np.ascontiguousarray(
                w_qkv[:, 2048 + 512 * g:2048 + 512 * g + 512]).astype(bf16),
            "wout": np.ascontiguousarray(w_out[cs, :]).astype(bf16),
        })
    return in_maps


def run_sharded(inputs, trace=False, trace_kwargs=None):
    """Run on 8 neuron cores; returns (out[B,S,D], BassKernelResults)."""
    from concourse import bass_utils

    with _lock:
        if "nc" not in _cache:
            _cache["nc"] = build_nc()
    nc = _cache["nc"]
    in_maps = make_in_maps(**inputs)
    res = bass_utils.run_bass_kernel_spmd(
        nc, in_maps, core_ids=list(range(NCORES)),
        trace=trace, **(trace_kwargs or {}),
    )
    outs = np.stack(
        [res.results[2 * b]["out"].astype(np.float32)
         + res.results[2 * b + 1]["out"].astype(np.float32)
         for b in range(B)]
    )
    return outs, res


def kernel(x, w_qkv, w_out):
    out, _ = run_sharded({"x": x, "w_qkv": w_qkv, "w_out": w_out})
    return out
